# revision 3
# baseline (speedup 1.0000x reference)
"""KAN layer (B-spline + silu) Trainium2 kernel.

Math: out[b,o] = sum_i w[i,o]*(silu(x[b,i]) + sum_k bx[b,i,k]*c[(i,o),k])
with bx the cubic B-spline basis on uniform knots from -5.25 to 5.25 (h=0.75).

Reformulation: with xs = (x - t0)/h, each basis function is
  bx_k(x) = (1/6) * sum_{j=0..4} (-1)^j C(4,j) * p_{k+j},   p_m = max(xs-m,0)^3
(truncated-power representation; p_14 == 0 for x < 5.25, which holds for the
data range, so m = 0..13 suffices). The banded 11x14 combination matrix is
folded into the weights, so the device computes a single dense matmul
  out[b,o] = sum_r A[r,b] * BW[r,o]
over r = 960 feature-rows per core: [silu(x_i) (64 rows); p_m(x_i) (14*64 rows)].

Sharding: 8-way over n_in (i). Each core gets 64 input features, computes a
partial (1024, 512) output; partials are summed on the host (reduce-gather).
"""

import os
import numpy as np
from contextlib import ExitStack

import concourse.bass as bass
import concourse.bacc as bacc
import concourse.tile as tile
from concourse import mybir
from concourse.bass_utils import run_bass_kernel_spmd

N_CORES = 8
BATCH, N_IN, N_OUT = 1024, 512, 512
I_SH = N_IN // N_CORES          # 64 input features per core
GRID, DEGREE = 8, 3
NB = GRID + DEGREE              # 11 basis functions
LO, HI = -3.0, 3.0
H = (HI - LO) / GRID            # 0.75
T0 = LO - DEGREE * H            # -5.25
INV_H = 1.0 / H
C0 = -T0 / H                    # xs = x*INV_H + C0
NM = 14                         # p_0 .. p_13
K_ROWS = I_SH * (1 + NM)        # 960 = 64 silu + 14*64 spline features
N_FULL = 7                      # full 128-row K chunks
# chunk 7 is the 64-row tail (p_13)

AluOp = mybir.AluOpType
Act = mybir.ActivationFunctionType

_CACHE = {}


def _fold_matrix():
    """11x14 matrix M with bx_k = sum_m M[k,m] * p_m."""
    coef = np.array([1.0, -4.0, 6.0, -4.0, 1.0]) / 6.0
    M = np.zeros((NB, NM))
    for k in range(NB):
        for j in range(5):
            if k + j < NM:
                M[k, k + j] = coef[j]
    return M


def _build_program():
    """Build + compile the SPMD single-core program (same on all 8 cores)."""
    use_f32r = os.environ.get("KAN_F32R", "1") == "1"
    mm_dt = mybir.dt.float32r if use_f32r else mybir.dt.float32

    nc = bacc.Bacc(
        "TRN2", target_bir_lowering=False, debug=False,
        enable_asserts=False, num_devices=N_CORES,
    )
    xT_d = nc.dram_tensor("xT", [I_SH, BATCH], mybir.dt.float32,
                          kind="ExternalInput").ap()
    bw_d = nc.dram_tensor("BW", [K_ROWS, N_OUT], mybir.dt.float32,
                          kind="ExternalInput").ap()
    con_d = nc.dram_tensor("CONSTS", [128, 8], mybir.dt.float32,
                           kind="ExternalInput").ap()
    out_d = nc.dram_tensor("OUT", [BATCH, N_OUT], mybir.dt.float32,
                           kind="ExternalOutput").ap()

    with tile.TileContext(nc) as tc, ExitStack() as ctx:
        pool = ctx.enter_context(tc.tile_pool(name="main", bufs=1))
        scratch = ctx.enter_context(tc.tile_pool(name="scratch", bufs=2))
        psum = ctx.enter_context(tc.tile_pool(name="psum", bufs=1, space="PSUM"))

        # ---- input DMAs -------------------------------------------------
        x_st = pool.tile([128, BATCH], mybir.dt.float32, name="x_st")
        nc.sync.dma_start(x_st[0:I_SH, :], xT_d[:])
        nc.sync.dma_start(x_st[I_SH:128, :], xT_d[:])
        consts = pool.tile([128, 8], mybir.dt.float32, name="consts")
        nc.sync.dma_start(consts[:], con_d[:])
        bw_t = []
        for cdx in range(8):
            rows = 128 if cdx < N_FULL else K_ROWS - 128 * N_FULL
            t = pool.tile([rows, N_OUT], mybir.dt.float32, name=f"bw{cdx}")
            nc.sync.dma_start(t[:], bw_d[128 * cdx:128 * cdx + rows, :])
            bw_t.append(t)

        # ---- feature production ----------------------------------------
        # xs = x*1/h + c0, stacked twice along partitions
        xs = pool.tile([128, BATCH], mybir.dt.float32, name="xs")
        nc.vector.tensor_scalar(xs[:], x_st[:], float(INV_H), float(C0),
                                AluOp.mult, AluOp.add)

        at = []
        for cdx in range(8):
            rows = 128 if cdx < N_FULL else K_ROWS - 128 * N_FULL
            at.append(pool.tile([rows, BATCH], mybir.dt.float32, name=f"at{cdx}"))

        # chunk 0: rows 0:64 silu = x*sigmoid(x), rows 64:128 p_0 = xs^3
        sg = scratch.tile([I_SH, BATCH], mybir.dt.float32, name="sg", tag="sq")
        nc.scalar.activation(sg[:], x_st[0:I_SH, :], Act.Sigmoid)
        nc.vector.tensor_mul(at[0][0:I_SH, :], sg[:], x_st[0:I_SH, :])
        s0 = scratch.tile([I_SH, BATCH], mybir.dt.float32, name="sq", tag="sq")
        nc.scalar.activation(s0[:], xs[0:I_SH, :], Act.Square)
        nc.vector.tensor_mul(at[0][I_SH:128, :], s0[:], xs[0:I_SH, :])

        # chunks 1..6: pairs (p_{2g-1}, p_{2g}); chunk 7: p_13 (64 rows)
        # consts[:, g-1] holds the per-partition m values for group g
        gpsimd_groups = {2, 5}          # offload some multiplies to GpSimd
        for g in range(1, 8):
            rows = 128 if g < 7 else I_SH
            y = scratch.tile([rows, BATCH], mybir.dt.float32, name=f"y{g}", tag="y")
            nc.vector.tensor_scalar(y[:], xs[0:rows, :],
                                    consts[0:rows, g - 1:g], 0.0,
                                    AluOp.subtract, AluOp.max)
            s = scratch.tile([rows, BATCH], mybir.dt.float32, name=f"s{g}", tag="sq")
            nc.scalar.activation(s[:], y[:], Act.Square)
            eng = nc.gpsimd if g in gpsimd_groups else nc.vector
            eng.tensor_mul(at[g][:], s[:], y[:])

        # ---- main matmul: out[b,o] += A[r,b]^T @ BW[r,o] ----------------
        ps = [psum.tile([128, N_OUT], mybir.dt.float32, name=f"ps{t}")
              for t in range(8)]
        for cdx in range(8):
            for t in range(8):
                nc.tensor.matmul(
                    ps[t][:],
                    at[cdx][:, bass.ts(t, 128)].bitcast(mm_dt),
                    bw_t[cdx][:].bitcast(mm_dt),
                    start=(cdx == 0), stop=(cdx == 7),
                )
        for t in range(8):
            ob = pool.tile([128, N_OUT], mybir.dt.float32, name=f"ob{t}")
            if t % 2 == 0:
                nc.scalar.copy(ob[:], ps[t][:])
            else:
                nc.vector.tensor_copy(ob[:], ps[t][:])
            nc.sync.dma_start(out_d[128 * t:128 * (t + 1), :], ob[:])

    nc.compile()
    return nc


def _host_prep(x, c, w):
    """Per-core input maps."""
    x = np.asarray(x, dtype=np.float32)
    c = np.asarray(c, dtype=np.float32)
    w = np.asarray(w, dtype=np.float32)
    xT = np.ascontiguousarray(x.T)                      # (512, 1024)
    cr = c.reshape(N_IN, N_OUT, NB)
    M = _fold_matrix()                                  # (11, 14) float64
    cwM = np.einsum("iok,km->iom",
                    (cr * w[:, :, None]).astype(np.float64), M).astype(np.float32)

    consts = np.zeros((128, 8), dtype=np.float32)
    for g in range(1, 7):                               # pairs (2g-1, 2g)
        consts[0:64, g - 1] = 2 * g - 1
        consts[64:128, g - 1] = 2 * g
    consts[0:64, 6] = 13.0                              # tail group

    in_maps = []
    for j in range(N_CORES):
        i0, i1 = j * I_SH, (j + 1) * I_SH
        bw = np.concatenate(
            [w[i0:i1],                                   # silu rows
             cwM[i0:i1].transpose(2, 0, 1).reshape(NM * I_SH, N_OUT)], axis=0)
        in_maps.append({
            "xT": np.ascontiguousarray(xT[i0:i1]),
            "BW": np.ascontiguousarray(bw),
            "CONSTS": consts,
        })
    return in_maps


def kernel(x, c, w):
    if "nc" not in _CACHE:
        _CACHE["nc"] = _build_program()
    nc = _CACHE["nc"]
    in_maps = _host_prep(x, c, w)
    res = run_bass_kernel_spmd(nc, in_maps, list(range(N_CORES)))
    _CACHE["last_results"] = res
    out = res.results[0]["OUT"].astype(np.float32)
    for j in range(1, N_CORES):
        out = out + res.results[j]["OUT"]
    return out.astype(np.float32)


# revision 5
# speedup vs baseline: 1.0044x; 1.0044x over previous
"""KAN layer (B-spline + silu) Trainium2 kernel.

Math: out[b,o] = sum_i w[i,o]*(silu(x[b,i]) + sum_k bx[b,i,k]*c[(i,o),k])
with bx the cubic B-spline basis on uniform knots from -5.25 to 5.25 (h=0.75).

Reformulation: with xs = (x - t0)/h, each basis function is
  bx_k(x) = (1/6) * sum_{j=0..4} (-1)^j C(4,j) * p_{k+j},   p_m = max(xs-m,0)^3
(truncated-power representation; p_14 == 0 for x < 5.25, which holds for the
data range, so m = 0..13 suffices). The banded 11x14 combination matrix is
folded into the weights, so the device computes a single dense matmul
  out[b,o] = sum_r A[r,b] * BW[r,o]
over r = 960 feature-rows per core: [silu(x_i) (64 rows); p_m(x_i) (14*64 rows)].

Sharding: 8-way over n_in (i). Each core gets 64 input features, computes a
partial (1024, 512) output; partials are summed on the host (reduce-gather).
"""

import os
import numpy as np
from contextlib import ExitStack

import concourse.bass as bass
import concourse.bacc as bacc
import concourse.tile as tile
from concourse import mybir
from concourse.bass_utils import run_bass_kernel_spmd

N_CORES = 8
BATCH, N_IN, N_OUT = 1024, 512, 512
I_SH = N_IN // N_CORES          # 64 input features per core
GRID, DEGREE = 8, 3
NB = GRID + DEGREE              # 11 basis functions
LO, HI = -3.0, 3.0
H = (HI - LO) / GRID            # 0.75
T0 = LO - DEGREE * H            # -5.25
INV_H = 1.0 / H
C0 = -T0 / H                    # xs = x*INV_H + C0
NM = 14                         # p_0 .. p_13
K_ROWS = I_SH * (1 + NM)        # 960 = 64 silu + 14*64 spline features
N_FULL = 7                      # full 128-row K chunks
# chunk 7 is the 64-row tail (p_13)

AluOp = mybir.AluOpType
Act = mybir.ActivationFunctionType

_CACHE = {}


def _fold_matrix():
    """11x14 matrix M with bx_k = sum_m M[k,m] * p_m."""
    coef = np.array([1.0, -4.0, 6.0, -4.0, 1.0]) / 6.0
    M = np.zeros((NB, NM))
    for k in range(NB):
        for j in range(5):
            if k + j < NM:
                M[k, k + j] = coef[j]
    return M


def _build_program():
    """Build + compile the SPMD single-core program (same on all 8 cores)."""
    use_f32r = os.environ.get("KAN_F32R", "1") == "1"
    mm_dt = mybir.dt.float32r if use_f32r else mybir.dt.float32

    nc = bacc.Bacc(
        "TRN2", target_bir_lowering=False, debug=False,
        enable_asserts=False, num_devices=N_CORES,
    )
    xT_d = nc.dram_tensor("xT", [I_SH, BATCH], mybir.dt.float32,
                          kind="ExternalInput").ap()
    bw_d = nc.dram_tensor("BW", [K_ROWS, N_OUT], mm_dt,
                          kind="ExternalInput").ap()
    con_d = nc.dram_tensor("CONSTS", [128, 8], mybir.dt.float32,
                           kind="ExternalInput").ap()
    out_d = nc.dram_tensor("OUT", [BATCH, N_OUT], mybir.dt.float32,
                           kind="ExternalOutput").ap()

    with tile.TileContext(nc) as tc, ExitStack() as ctx:
        pool = ctx.enter_context(tc.tile_pool(name="main", bufs=1))
        scratch = ctx.enter_context(tc.tile_pool(name="scratch", bufs=2))
        psum = ctx.enter_context(tc.tile_pool(name="psum", bufs=1, space="PSUM"))

        # ---- input DMAs -------------------------------------------------
        x_st = pool.tile([128, BATCH], mybir.dt.float32, name="x_st")
        nc.sync.dma_start(x_st[0:I_SH, :], xT_d[:])
        nc.sync.dma_start(x_st[I_SH:128, :], xT_d[:])
        consts = pool.tile([128, 8], mybir.dt.float32, name="consts")
        nc.sync.dma_start(consts[:], con_d[:])
        bw_t = []
        for cdx in range(8):
            rows = 128 if cdx < N_FULL else K_ROWS - 128 * N_FULL
            t = pool.tile([rows, N_OUT], mm_dt, name=f"bw{cdx}")
            nc.sync.dma_start(t[:], bw_d[128 * cdx:128 * cdx + rows, :])
            bw_t.append(t)

        # ---- feature production ----------------------------------------
        # xs = x*1/h + c0, stacked twice along partitions
        xs = pool.tile([128, BATCH], mybir.dt.float32, name="xs")
        nc.vector.tensor_scalar(xs[:], x_st[:], float(INV_H), float(C0),
                                AluOp.mult, AluOp.add)

        at = []
        for cdx in range(8):
            rows = 128 if cdx < N_FULL else K_ROWS - 128 * N_FULL
            at.append(pool.tile([rows, BATCH], mm_dt, name=f"at{cdx}"))

        # chunk 0: rows 0:64 silu = x*sigmoid(x), rows 64:128 p_0 = xs^3
        sg = scratch.tile([I_SH, BATCH], mybir.dt.float32, name="sg", tag="sq")
        nc.scalar.activation(sg[:], x_st[0:I_SH, :], Act.Sigmoid)
        nc.vector.tensor_mul(at[0][0:I_SH, :], sg[:], x_st[0:I_SH, :])
        s0 = scratch.tile([I_SH, BATCH], mybir.dt.float32, name="sq", tag="sq")
        nc.scalar.activation(s0[:], xs[0:I_SH, :], Act.Square)
        nc.vector.tensor_mul(at[0][I_SH:128, :], s0[:], xs[0:I_SH, :])

        # chunks 1..6: pairs (p_{2g-1}, p_{2g}); chunk 7: p_13 (64 rows)
        # consts[:, g-1] holds the per-partition m values for group g
        gpsimd_groups = {2, 5}          # offload some multiplies to GpSimd
        for g in range(1, 8):
            rows = 128 if g < 7 else I_SH
            y = scratch.tile([rows, BATCH], mybir.dt.float32, name=f"y{g}", tag="y")
            nc.vector.tensor_scalar(y[:], xs[0:rows, :],
                                    consts[0:rows, g - 1:g], 0.0,
                                    AluOp.subtract, AluOp.max)
            s = scratch.tile([rows, BATCH], mybir.dt.float32, name=f"s{g}", tag="sq")
            nc.scalar.activation(s[:], y[:], Act.Square)
            eng = nc.gpsimd if g in gpsimd_groups else nc.vector
            eng.tensor_mul(at[g][:], s[:], y[:])

        # ---- main matmul: out[b,o] += A[r,b]^T @ BW[r,o] ----------------
        ps = [psum.tile([128, N_OUT], mybir.dt.float32, name=f"ps{t}")
              for t in range(8)]
        for cdx in range(8):
            for t in range(8):
                nc.tensor.matmul(
                    ps[t][:],
                    at[cdx][:, bass.ts(t, 128)],
                    bw_t[cdx][:],
                    start=(cdx == 0), stop=(cdx == 7),
                )
        for t in range(8):
            ob = pool.tile([128, N_OUT], mybir.dt.float32, name=f"ob{t}")
            if t % 2 == 0:
                nc.scalar.copy(ob[:], ps[t][:])
            else:
                nc.vector.tensor_copy(ob[:], ps[t][:])
            nc.sync.dma_start(out_d[128 * t:128 * (t + 1), :], ob[:])

    nc.compile()
    return nc


def _host_prep(x, c, w):
    """Per-core input maps."""
    x = np.asarray(x, dtype=np.float32)
    c = np.asarray(c, dtype=np.float32)
    w = np.asarray(w, dtype=np.float32)
    xT = np.ascontiguousarray(x.T)                      # (512, 1024)
    cr = c.reshape(N_IN, N_OUT, NB)
    M = _fold_matrix()                                  # (11, 14) float64
    cwM = np.einsum("iok,km->iom",
                    (cr * w[:, :, None]).astype(np.float64), M).astype(np.float32)

    consts = np.zeros((128, 8), dtype=np.float32)
    for g in range(1, 7):                               # pairs (2g-1, 2g)
        consts[0:64, g - 1] = 2 * g - 1
        consts[64:128, g - 1] = 2 * g
    consts[0:64, 6] = 13.0                              # tail group

    in_maps = []
    for j in range(N_CORES):
        i0, i1 = j * I_SH, (j + 1) * I_SH
        bw = np.concatenate(
            [w[i0:i1],                                   # silu rows
             cwM[i0:i1].transpose(2, 0, 1).reshape(NM * I_SH, N_OUT)], axis=0)
        in_maps.append({
            "xT": np.ascontiguousarray(xT[i0:i1]),
            "BW": np.ascontiguousarray(bw),
            "CONSTS": consts,
        })
    return in_maps


def kernel(x, c, w):
    if "nc" not in _CACHE:
        _CACHE["nc"] = _build_program()
    nc = _CACHE["nc"]
    in_maps = _host_prep(x, c, w)
    res = run_bass_kernel_spmd(nc, in_maps, list(range(N_CORES)))
    _CACHE["last_results"] = res
    out = res.results[0]["OUT"].astype(np.float32)
    for j in range(1, N_CORES):
        out = out + res.results[j]["OUT"]
    return out.astype(np.float32)
